# revision 23
# baseline (speedup 1.0000x reference)
"""Trainium2 Bass kernel for MultiHeadAttention (B=2, S=2048, D=1024, H=16).

Returns (normed, attn) like the reference nn.Module:
  q/k/v projections -> scaled dot-product attention (full [B,H,S,S] softmax
  matrix is an output) -> output projection -> residual with projected q ->
  LayerNorm.

Sharding over 8 NeuronCores: batch (2) x head-groups (4 heads per core).
After the output projection, a 4-core ReduceScatter(add) combines the
per-head-group partial sums; each core LayerNorms its 512-row shard.

All matmuls run as float32r (full-rate fp32 PE mode, ~1e-4 rel err).
Masking is folded into the score matmuls by augmenting qT with a ones row
and kT with an additive-mask row (K=65). Softmax normalization of the attn
output is folded into the path-B exp as a -ln(sum) per-partition bias.
"""

import numpy as np

import concourse.bass as bass
import concourse.mybir as mybir
import concourse.tile as tile
from concourse import bacc
from concourse import bass_utils

F32 = mybir.dt.float32
F32R = mybir.dt.float32r

B, S, D, H = 2, 2048, 1024, 16
DK = D // H          # 64
HPC = 4              # heads per core
CPB = 4              # cores per batch group
COLS = HPC * DK      # 256 local columns
NKC = D // 128       # 8 contraction chunks
NST = S // 128       # 16 s-tiles
SCALE = 1.0 / np.sqrt(np.float32(DK))  # 1/8
MASK_NEG = np.float32(-1.0e9)
LN_EPS = 1e-5


def build_program():
    nc = bacc.Bacc(num_devices=8)

    # ---------------- DRAM I/O ----------------
    xqt = nc.dram_tensor("xqt", [NKC, 128, S], F32R, kind="ExternalInput")
    xkt = nc.dram_tensor("xkt", [NKC, 128, S], F32R, kind="ExternalInput")
    xvt = nc.dram_tensor("xvt", [NKC, 128, S], F32R, kind="ExternalInput")
    wq = nc.dram_tensor("wq", [NKC, 128, COLS], F32R, kind="ExternalInput")
    wk = nc.dram_tensor("wk", [NKC, 128, COLS], F32R, kind="ExternalInput")
    wv = nc.dram_tensor("wv", [NKC, 128, COLS], F32R, kind="ExternalInput")
    wo = nc.dram_tensor("wo", [2, 128, D], F32R, kind="ExternalInput")
    jsel = nc.dram_tensor("jsel", [64, 4 * D], F32R, kind="ExternalInput")
    bqk = nc.dram_tensor("bqk", [128, 4], F32, kind="ExternalInput")
    bv256 = nc.dram_tensor("bv256", [128, COLS], F32, kind="ExternalInput")
    maskrow = nc.dram_tensor("maskrow", [1, S], F32R, kind="ExternalInput")
    boadd = nc.dram_tensor("boadd", [128, D], F32, kind="ExternalInput")
    gamma128 = nc.dram_tensor("gamma128", [128, D], F32, kind="ExternalInput")
    beta128 = nc.dram_tensor("beta128", [128, D], F32, kind="ExternalInput")

    attn_part = nc.dram_tensor("attn_part", [HPC, S, S], F32, kind="ExternalOutput")
    normed_part = nc.dram_tensor(
        "normed_part", [S // CPB, D], F32, kind="ExternalOutput"
    )

    # internal DRAM
    cc_in = nc.dram_tensor("cc_in", [S, D], F32)
    cc_out = nc.dram_tensor("cc_out", [S // CPB, D], F32)
    srow = nc.dram_tensor("srow", [S], F32)
    rrow = nc.dram_tensor("rrow", [S], F32)

    from contextlib import ExitStack

    with tile.TileContext(nc) as tc, ExitStack() as stack:
        persist = stack.enter_context(tc.tile_pool(name="persist", bufs=1))

        # ---- persistent activations ----
        # qTh/kTh: [65, S] per head (row 64: ones for q / additive mask for k)
        qTh = [persist.tile([65, S], F32R, tag=f"qTh{h}", name=f"qTh{h}") for h in range(HPC)]
        kTh = [persist.tile([65, S], F32R, tag=f"kTh{h}", name=f"kTh{h}") for h in range(HPC)]
        # paired layout of qT for the residual selection matmul
        # ctx^T normalized, heads paired: [128, S] x 2
        ctxT = [persist.tile([128, S], F32R, tag=f"ctxT{m}", name=f"ctxT{m}") for m in range(2)]
        # per-head -ln(softmax denominators), [sq-partition, tile] layout
        negln_p = [persist.tile([128, NST], F32, tag=f"negln{h}", name=f"negln{h}")
                   for h in range(HPC)]
        vvpool = stack.enter_context(tc.tile_pool(name="vvpool", bufs=1))
        vv = [vvpool.tile([128, NST, DK + 1], F32R, tag=f"vv{h}", name=f"vv{h}") for h in range(HPC)]

        # ================= P1: projections =================
        # q and k (transposed layout), m-chunk = 2 heads each
        with tc.tile_pool(name="pqk", bufs=2, space="PSUM") as pqk, tc.tile_pool(
            name="xs", bufs=3
        ) as xs, tc.tile_pool(name="w1", bufs=1) as w1:
            wq_sb = w1.tile([128, NKC, COLS], F32R, tag="wq")
            wk_sb = w1.tile([128, NKC, COLS], F32R, tag="wk")
            for kc in range(NKC):
                nc.sync.dma_start(wq_sb[:, kc, :], wq[kc])
                nc.sync.dma_start(wk_sb[:, kc, :], wk[kc])
            bqk_sb = w1.tile([128, 4], F32, tag="bqk")
            nc.sync.dma_start(bqk_sb[:], bqk.ap())
            ones32 = w1.tile([1, S], F32, tag="ones32")
            nc.vector.memset(ones32[:], 1.0)
            for which, xdram, w_sb, bcol0, dst_pair in (
                ("q", xqt, wq_sb, 0, True),
                ("k", xkt, wk_sb, 2, False),
            ):
                pm = [pqk.tile([128, S], F32, tag="pqk", name=f"pm{_}") for _ in range(2)]
                for kc in range(NKC):
                    xt = xs.tile([128, S], F32R, tag="xs")
                    nc.sync.dma_start(xt[:], xdram[kc])
                    for m in range(2):
                        for n in range(4):
                            nc.tensor.matmul(
                                pm[m][:, n * 512 : (n + 1) * 512],
                                (w_sb[:, kc, m * 128 : (m + 1) * 128]),
                                (xt[:, n * 512 : (n + 1) * 512]),
                                start=(kc == 0),
                                stop=(kc == NKC - 1),
                            )
                for m in range(2):
                    for j in range(2):
                        h = 2 * m + j
                        dst = qTh[h] if dst_pair else kTh[h]
                        nc.vector.tensor_scalar_add(
                            dst[0:64, :],
                            pm[m][64 * j : 64 * j + 64, :],
                            bqk_sb[64 * j : 64 * j + 64, bcol0 + m : bcol0 + m + 1],
                        )
                        if dst_pair:
                            nc.vector.tensor_copy(dst[64:65, :], ones32[:])
                        else:
                            nc.sync.dma_start(dst[64:65, :], maskrow.ap())

        # v projection (natural layout), streamed in two s-halves
        with tc.tile_pool(name="pv", bufs=1, space="PSUM") as pvp, tc.tile_pool(
            name="xsv", bufs=3
        ) as xsv, tc.tile_pool(name="w2", bufs=1) as w2:
            wv_sb = w2.tile([128, NKC, COLS], F32R, tag="wv")
            for kc in range(NKC):
                nc.sync.dma_start(wv_sb[:, kc, :], wv[kc])
            bv_sb = w2.tile([128, COLS], F32, tag="bv")
            nc.sync.dma_start(bv_sb[:], bv256.ap())
            onesv = w2.tile([128, NST * (DK + 1)], F32, tag="onesv")
            nc.vector.memset(onesv[:], 1.0)
            for h in range(HPC):
                nc.vector.tensor_copy(
                    vv[h][:].rearrange("p a b -> p (a b)"), onesv[:]
                )
            for half in range(2):
                hs = slice(half * 1024, (half + 1) * 1024)
                pvs = [pvp.tile([128, COLS], F32, tag=f"pv{i}", name=f"pv{i}")
                       for i in range(8)]
                for kc in range(NKC):
                    xt = xsv.tile([128, 1024], F32R, tag="xsv")
                    nc.sync.dma_start(xt[:], xvt[kc][:, hs])
                    for i in range(8):
                        nc.tensor.matmul(
                            pvs[i][:],
                            (xt[:, i * 128 : (i + 1) * 128]),
                            (wv_sb[:, kc, :]),
                            start=(kc == 0),
                            stop=(kc == NKC - 1),
                        )
                for i in range(8):
                    st = half * 8 + i
                    for h in range(HPC):
                        nc.vector.tensor_add(
                            vv[h][:, st, 0:DK],
                            pvs[i][:, h * DK : (h + 1) * DK],
                            bv_sb[:, h * DK : (h + 1) * DK],
                        )

        # ===== P2/P3: interleaved A/B slot pipeline, dedicated PSUM partitions =====
        with tc.tile_pool(name="pw", bufs=3, space="PSUM") as pw, tc.tile_pool(
            name="pwb", bufs=1, space="PSUM"
        ) as pwb, tc.tile_pool(name="etp", bufs=6) as etp, tc.tile_pool(
            name="ctxa", bufs=1
        ) as ctxa, tc.tile_pool(name="sumsp", bufs=1) as sumsp, tc.tile_pool(
            name="attnst", bufs=4
        ) as attnst, tc.tile_pool(name="w3", bufs=1) as w3, tc.tile_pool(
            name="osb", bufs=2
        ) as osbp, tc.tile_pool(name="lnst", bufs=2) as lnst:
            sums_p = sumsp.tile([128, NST], F32, tag="sums_p")
            recip_p = sumsp.tile([128, NST], F32, tag="recip_p")
            recip_rep = sumsp.tile([64, S], F32, tag="recip_rep")
            ctx_acc = ctxa.tile([65, S], F32, tag="ctx_acc")
            wo_sb = w3.tile([128, 2, D], F32R, tag="wo")
            for cc in range(2):
                nc.sync.dma_start(wo_sb[:, cc, :], wo[cc])
            jsel_sb = w3.tile([64, 4, D], F32R, tag="jsel")
            nc.sync.dma_start(jsel_sb[:], jsel.ap().rearrange("p (h d) -> p h d", h=4))
            boadd_sb = w3.tile([128, D], F32, tag="boadd")
            nc.sync.dma_start(boadd_sb[:], boadd.ap())
            gamma_sb = w3.tile([128, D], F32, tag="gamma")
            nc.sync.dma_start(gamma_sb[:], gamma128.ap())
            beta_sb = w3.tile([128, D], F32, tag="beta")
            nc.sync.dma_start(beta_sb[:], beta128.ap())
            eps_sb = w3.tile([128, 1], F32, tag="eps")
            nc.vector.memset(eps_sb[:], LN_EPS)

            cp_box = [None, None]

            def emit_a_tile(h, st, half):
                hs = slice(half * 1024, (half + 1) * 1024)
                psc = pw.tile([128, 1024], F32, tag="w", name=f"psc{h}_{st}_{half}")
                for n in range(2):
                    nc.tensor.matmul(
                        psc[:, n * 512 : (n + 1) * 512],
                        kTh[h][:, st * 128 : (st + 1) * 128],
                        qTh[h][:, half * 1024 + n * 512 : half * 1024 + (n + 1) * 512],
                        start=True,
                        stop=True,
                    )
                et = etp.tile([128, 1024], F32R, tag="et", name=f"et{h}_{st}_{half}")
                nc.scalar.activation(
                    out=et[:], in_=psc[:],
                    func=mybir.ActivationFunctionType.Exp, scale=float(SCALE),
                )
                if st % 2 == 0:
                    cp_box[half] = pw.tile(
                        [65, 1024], F32, tag="w", name=f"cp{h}_{st}_{half}"
                    )
                cp = cp_box[half]
                for n in range(2):
                    nc.tensor.matmul(
                        cp[:, n * 512 : (n + 1) * 512],
                        vv[h][:, st, :],
                        et[:, n * 512 : (n + 1) * 512],
                        start=(st % 2 == 0),
                        stop=(st % 2 == 1),
                    )
                if st == 1:
                    nc.vector.tensor_copy(ctx_acc[:, hs], cp[:])
                elif st % 2 == 1:
                    nc.vector.tensor_add(ctx_acc[:, hs], ctx_acc[:, hs], cp[:])

            def emit_sums(h):
                nc.sync.dma_start(srow.ap(), ctx_acc[64:65, :])
                nc.gpsimd.dma_start(
                    sums_p[:], srow.ap().rearrange("(t p) -> p t", p=128)
                )
                nc.vector.reciprocal(recip_p[:], sums_p[:])
                nc.scalar.activation(
                    out=negln_p[h][:], in_=sums_p[:],
                    func=mybir.ActivationFunctionType.Ln,
                )
                nc.vector.tensor_scalar_mul(negln_p[h][:], negln_p[h][:], -1.0)
                nc.gpsimd.dma_start(
                    rrow.ap().rearrange("(t p) -> p t", p=128), recip_p[:]
                )
                nc.gpsimd.dma_start(recip_rep[:], rrow.ap().partition_broadcast(64))
                nc.vector.tensor_mul(
                    ctxT[h // 2][64 * (h % 2) : 64 * (h % 2) + 64, :],
                    ctx_acc[0:64, :],
                    recip_rep[:],
                )

            def emit_b_tile(h, sq, half):
                pb = pwb.tile([128, 1024], F32, tag="b", name=f"pb{h}_{sq}_{half}")
                for n in range(2):
                    nc.tensor.matmul(
                        pb[:, n * 512 : (n + 1) * 512],
                        qTh[h][:, sq * 128 : (sq + 1) * 128],
                        kTh[h][:, half * 1024 + n * 512 : half * 1024 + (n + 1) * 512],
                        start=True,
                        stop=True,
                    )
                at = attnst.tile([128, 1024], F32, tag="at", name=f"at{h}_{sq}_{half}")
                nc.scalar.activation(
                    out=at[:], in_=pb[:],
                    func=mybir.ActivationFunctionType.Exp,
                    bias=negln_p[h][:, sq : sq + 1],
                    scale=float(SCALE),
                )
                nc.sync.dma_start(
                    attn_part[
                        h, sq * 128 : (sq + 1) * 128,
                        half * 1024 : (half + 1) * 1024,
                    ],
                    at[:],
                )

            def emit_outproj_tile(stl):
                po = pw.tile([128, D], F32, tag="w", name=f"po{stl}")
                sl = slice(stl * 128, (stl + 1) * 128)
                for n in range(2):
                    ns = slice(n * 512, (n + 1) * 512)
                    ops = [(ctxT[0][:, sl], wo_sb[:, 0, ns]),
                           (ctxT[1][:, sl], wo_sb[:, 1, ns])]
                    ops += [(qTh[h][0:64, sl], jsel_sb[:, h, ns]) for h in range(HPC)]
                    for oi, (lhs, rhs) in enumerate(ops):
                        nc.tensor.matmul(
                            po[:, ns], lhs, rhs,
                            start=(oi == 0), stop=(oi == len(ops) - 1),
                        )
                o_sb = osbp.tile([128, D], F32, tag="osb", name=f"osb{stl}")
                nc.vector.tensor_add(o_sb[:], po[:], boadd_sb[:])
                nc.sync.dma_start(cc_in[sl, :], o_sb[:])

            for slot in range(HPC + 1):
                ha, hb = slot, slot - 1
                if hb == HPC - 1:
                    for i in range(NST):
                        emit_outproj_tile(i)
                for i in range(NST):
                    for half in range(2):
                        if ha < HPC:
                            emit_a_tile(ha, i, half)
                        if hb >= 0:
                            emit_b_tile(hb, i, half)
                if ha < HPC:
                    emit_sums(ha)

            for rs_half in range(2):
                nc.gpsimd.collective_compute(
                    "ReduceScatter",
                    mybir.AluOpType.add,
                    ins=[cc_in[rs_half * 1024 : (rs_half + 1) * 1024, :].opt()],
                    outs=[cc_out[rs_half * 256 : (rs_half + 1) * 256, :].opt()],
                    replica_groups=[[0, 1, 2, 3], [4, 5, 6, 7]],
                )

            for t in range(4):
                sl = slice(t * 128, (t + 1) * 128)
                lt = osbp.tile([128, D], F32, tag="osb", name=f"lt{t}")
                nc.sync.dma_start(lt[:], cc_out[sl, :])
                stats = lnst.tile([128, 2, nc.vector.BN_STATS_DIM], F32, tag="stats")
                for sgi in range(2):
                    nc.vector.bn_stats(
                        out=stats[:, sgi, :], in_=lt[:, sgi * 512 : (sgi + 1) * 512]
                    )
                mv = lnst.tile([128, nc.vector.BN_AGGR_DIM], F32, tag="mv")
                nc.vector.bn_aggr(out=mv[:], in_=stats[:])
                std = lnst.tile([128, 1], F32, tag="std")
                nc.scalar.activation(
                    out=std[:], in_=mv[:, 1:2],
                    func=mybir.ActivationFunctionType.Sqrt, bias=eps_sb[:],
                )
                nc.vector.reciprocal(std[:], std[:])
                nc.vector.tensor_scalar(
                    out=lt[:], in0=lt[:],
                    scalar1=mv[:, 0:1], scalar2=std[:],
                    op0=mybir.AluOpType.subtract, op1=mybir.AluOpType.mult,
                )
                nc.vector.tensor_mul(lt[:], lt[:], gamma_sb[:])
                nc.vector.tensor_add(lt[:], lt[:], beta_sb[:])
                nc.sync.dma_start(normed_part[sl, :], lt[:])

    nc.finalize()
    return nc


def shard_inputs(Q, K, V, mask, Wq, bq, Wk, bk, Wv, bv, Wo, bo, gamma, beta):
    """Build the 8 per-core input maps (host-side layout only)."""
    in_maps = []
    maskadd = np.where(mask[:, 0, 0, :] == 0, MASK_NEG, np.float32(0.0)).astype(
        np.float32
    )  # [B, S]
    for c in range(8):
        b, g = divmod(c, CPB)
        cols = slice(g * COLS, (g + 1) * COLS)
        jsel_np = np.zeros((64, 4, D), np.float32)
        for h in range(4):
            for i in range(64):
                jsel_np[i, h, g * COLS + h * 64 + i] = 1.0
        jsel_np = jsel_np.reshape(64, 4 * D)
        bqk_np = np.stack(
            [
                bq[cols][0:128],
                bq[cols][128:256],
                bk[cols][0:128],
                bk[cols][128:256],
            ],
            axis=1,
        ).astype(np.float32)
        in_maps.append(
            {
                "xqt": np.ascontiguousarray(Q[b].T).reshape(NKC, 128, S),
                "xkt": np.ascontiguousarray(K[b].T).reshape(NKC, 128, S),
                "xvt": np.ascontiguousarray(V[b].T).reshape(NKC, 128, S),
                "wq": np.ascontiguousarray(Wq[:, cols]).reshape(NKC, 128, COLS),
                "wk": np.ascontiguousarray(Wk[:, cols]).reshape(NKC, 128, COLS),
                "wv": np.ascontiguousarray(Wv[:, cols]).reshape(NKC, 128, COLS),
                "wo": np.ascontiguousarray(Wo[cols, :]).reshape(2, 128, D),
                "jsel": jsel_np,
                "bqk": bqk_np,
                "bv256": np.tile(bv[cols][None, :], (128, 1)).astype(np.float32),
                "maskrow": maskadd[b][None, :],
                "boadd": (
                    np.tile(bo[None, :], (128, 1)).astype(np.float32)
                    if g == 0
                    else np.zeros((128, D), np.float32)
                ),
                "gamma128": np.tile(gamma[None, :], (128, 1)).astype(np.float32),
                "beta128": np.tile(beta[None, :], (128, 1)).astype(np.float32),
            }
        )
    return in_maps


_cached_nc = None


def kernel(Q, K, V, mask, Wq, bq, Wk, bk, Wv, bv, Wo, bo, gamma, beta, trace=False):
    global _cached_nc
    args = [np.asarray(a) for a in (Q, K, V, mask, Wq, bq, Wk, bk, Wv, bv, Wo, bo,
                                    gamma, beta)]
    Q, K, V, mask, Wq, bq, Wk, bk, Wv, bv, Wo, bo, gamma, beta = args
    in_maps = shard_inputs(Q, K, V, mask, Wq, bq, Wk, bk, Wv, bv, Wo, bo, gamma, beta)
    if _cached_nc is None:
        _cached_nc = build_program()
    nc = _cached_nc
    res = bass_utils.run_bass_kernel_spmd(
        nc, in_maps, core_ids=list(range(8)), trace=trace
    )
    attn = np.empty((B, H, S, S), np.float32)
    normed = np.empty((B, S, D), np.float32)
    for c in range(8):
        b, g = divmod(c, CPB)
        attn[b, g * HPC : (g + 1) * HPC] = res.results[c]["attn_part"]
        np_part = res.results[c]["normed_part"]
        # two quarter-shards from the two half-ReduceScatters
        normed[b, 256 * g : 256 * g + 256] = np_part[0:256]
        normed[b, 1024 + 256 * g : 1024 + 256 * g + 256] = np_part[256:512]
    kernel.last_exec_time_ns = res.exec_time_ns
    kernel.last_results = res
    return (normed, attn)


kernel.last_exec_time_ns = None
kernel.last_results = None
